# revision 26
# baseline (speedup 1.0000x reference)
"""GCN (2-layer + BN + global mean pool + sigmoid readout) on 8 TRN2 NeuronCores.

Strategy (see spec sharding_hint): destinations (nodes) sharded across the 8
cores; each core aggregates messages for its node shard.  Per layer:

  y = dinv * (X @ W)            (node-major, bf16, exchanged via AllGather)
  agg[c] = dinv[c] * sum_{e: col_e==c} y[row_e]        (+ self loop)
  h = relu(BN(agg))

The gather of y[row_e] uses bulk SWDGE dma_gather instructions (int16 indices,
128-edge chunks land edge-major on partitions).  The per-iteration time is
dominated by these gathers (~3.7 ns/descriptor, 256 B each), so the plan
minimizes descriptor count and maximizes DMA-queue concurrency:

  - edges are packed densely into chunks per (512-dest block, lo/hi source
    half) with no per-window padding (fill ~0.96);
  - self-loops are never gathered: one identity matmul per dest tile against
    the SBUF-resident node-major y (y_keep) adds them into PSUM;
  - slots are sorted by (dest-tile j, source id) so gather addresses ascend;
  - each group's gather is split in two instructions round-robined over the
    4 SWDGE queues (per-queue concurrency raises the descriptor rate ~35%).

The segment-sum is a TensorE matmul of each gathered chunk against a one-hot
S matrix built on DVE (is_equal of a dest map A2 vs iota); a chunk spanning
several dest tiles gets one matmul per (chunk, tile) pair.  PSUM evacuation
fuses the dinv scale (ACT), and each evacuated tile is immediately transposed
to feature-major (h_raw) while BN statistics accumulate (ACT accum), so after
the small stats AllReduce a single fused scale+bias+relu activation produces
the next layer's input.  The instruction stream is identical on all 8 cores
(SPMD); all per-core variation lives in input data (indices, A2, padding).
"""

import numpy as np
import ml_dtypes

import concourse.bacc as bacc
import concourse.bass as bass
import concourse.tile as tile
from concourse import mybir
from concourse.bass_utils import run_bass_kernel_spmd

BF16 = ml_dtypes.bfloat16
P = 128          # partitions / chunk size
BLOCK_DESTS = 512  # dests per PSUM block (4 tiles of 128)
EPS = 1e-5


class Dims:
    def __init__(self, N=50000, E=800000, F=96, H=128, G=64, C=50,
                 ncores=8, half=32768):
        assert N % ncores == 0
        self.N, self.E, self.F, self.H = N, E, F, H
        self.G, self.C = G, C
        self.ncores = ncores
        self.shard = N // ncores
        self.half = half                      # int16 gather range split
        self.ntile = ceil_div(self.shard, P)  # node tiles per shard
        self.shard_pad = self.ntile * P       # padded shard rows in y table
        self.npad = self.shard_pad * ncores   # padded y_full rows
        self.nblk = ceil_div(self.shard, BLOCK_DESTS)


def ceil_div(a, b):
    return (a + b - 1) // b


# ----------------------------------------------------------------------------
# Host planning: pure index/graph preprocessing (functions of edge_index/batch)
# ----------------------------------------------------------------------------

class Plan:
    pass


def make_plan(d: Dims, edge_index: np.ndarray, batch: np.ndarray) -> Plan:
    """Dense-packed chunk plan.

    Self-loops are NOT gathered (handled by identity matmuls against the
    SBUF-resident node-major y).  Per (dest-block of 512, src-half) the real
    edges are packed densely into 128-slot chunks, sorted by (dest-128-tile j,
    source) so gather addresses ascend.  A chunk may span several j groups;
    each (chunk, j) pair becomes one segment matmul whose one-hot S column is
    built from A2 (value = dest%128, or 999 for slots outside the j group).
    The (chunk, j) set is unioned across cores so the instruction stream is
    SPMD-identical; cores lacking a pair get an all-999 column (zero S).
    """
    pl = Plan()
    N, E = d.N, d.E
    rows_e = edge_index[0].astype(np.int64)
    cols_e = edge_index[1].astype(np.int64)
    # degree includes the self loop, as in the reference (edges + loops)
    deg = np.bincount(np.concatenate([cols_e, np.arange(N)]),
                      minlength=N).astype(np.float64)
    dinv = (1.0 / np.sqrt(np.maximum(deg, 1.0))).astype(np.float32)

    core_of = cols_e // d.shard
    # remap source node id to its padded position in the all-gathered y table
    pid = (rows_e // d.shard) * d.shard_pad + (rows_e % d.shard)
    nblk = d.nblk

    # per (core, block, half): slots sorted by (j, source)
    lists = {}
    for k in range(d.ncores):
        m = core_of == k
        r = pid[m]
        c = cols_e[m] - k * d.shard
        b = c // BLOCK_DESTS
        dl = c % BLOCK_DESTS
        j = dl // P
        hf = (r >= d.half).astype(np.int64)
        for bb in range(nblk):
            for h in (0, 1):
                sel = (b == bb) & (hf == h)
                rj, dlj, jj = r[sel], dl[sel], j[sel]
                order = np.lexsort((rj, jj))
                lists[(k, bb, h)] = (rj[order] - (d.half if h else 0),
                                    dlj[order], jj[order])

    # chunk counts per (block, half): max over cores
    nch = {}
    for bb in range(nblk):
        for h in (0, 1):
            mx = max(len(lists[(k, bb, h)][0]) for k in range(d.ncores))
            nch[(bb, h)] = ceil_div(mx, P)

    gstart = {}
    tot_slots = 0
    for bb in range(nblk):
        for h in (0, 1):
            gstart[(bb, h)] = tot_slots
            tot_slots += nch[(bb, h)] * P

    # segment-matmul union across cores: keys (block, half, chunk, j)
    mm_keys = set()
    for (k, bb, h), (r, dl, j) in lists.items():
        for c in range(ceil_div(len(r), P)):
            for jj in np.unique(j[c * P:(c + 1) * P]):
                mm_keys.add((bb, h, c, int(jj)))
    mm_list = sorted(mm_keys)
    acol_of = {key: i for i, key in enumerate(mm_list)}
    n_mm = len(mm_list)
    block_mms = [[] for _ in range(nblk)]
    for (bb, h, c, jj) in mm_list:
        block_mms[bb].append((h, c, jj, acol_of[(bb, h, c, jj)]))

    idx_all = np.zeros((d.ncores, tot_slots), dtype=np.int16)
    A2_all = np.full((d.ncores, n_mm, P), 999.0, dtype=np.float32)
    for (k, bb, h), (r, dl, j) in lists.items():
        n = len(r)
        g0 = gstart[(bb, h)]
        idx_all[k, g0:g0 + n] = r.astype(np.int16)
        for c in range(ceil_div(n, P)):
            sl = slice(c * P, min((c + 1) * P, n))
            jc, dlc = j[sl], dl[sl]
            for jj in np.unique(jc):
                acol = acol_of[(bb, h, c, int(jj))]
                colvals = np.full(P, 999.0, np.float32)
                view = colvals[:len(jc)]
                mask = jc == jj
                view[mask] = (dlc[mask] % P).astype(np.float32)
                A2_all[k, acol] = colvals

    # wrap idx to the [128, tot_slots//16] layout dma_gather wants:
    # slot i -> [16*c + i%16, i//16] for every q7 core c
    S16 = tot_slots // 16
    idx_wrapped = np.zeros((d.ncores, P, S16), dtype=np.int16)
    for k in range(d.ncores):
        w16 = idx_all[k].reshape(S16, 16).T  # [16, S16]
        idx_wrapped[k] = np.tile(w16, (8, 1))

    # A matrix in [128 partitions=slot%128, n_mm] layout
    A_pt = np.transpose(A2_all, (0, 2, 1)).astype(BF16)  # [cores, 128, n_mm]

    # per-core node-major helper arrays
    dinv_pt = np.zeros((d.ncores, P, d.ntile), dtype=np.float32)
    pool_pt = np.zeros((d.ncores, P, d.ntile, d.G), dtype=np.float32)
    for k in range(d.ncores):
        base = k * d.shard
        for t in range(d.ntile):
            for p in range(P):
                n0 = t * P + p
                if n0 < d.shard:
                    dinv_pt[k, p, t] = dinv[base + n0]
                    pool_pt[k, p, t, batch[base + n0]] = 1.0

    cnts = np.bincount(batch, minlength=d.G).astype(np.float32)
    inv_cnt = (1.0 / np.maximum(cnts, 1.0)).reshape(d.G, 1)

    pl.nch, pl.gstart, pl.tot_slots = nch, gstart, tot_slots
    pl.n_mm, pl.block_mms = n_mm, block_mms
    pl.idx_wrapped, pl.A_pt = idx_wrapped, A_pt
    pl.dinv_pt, pl.pool_pt, pl.inv_cnt = dinv_pt, pool_pt.reshape(d.ncores, P, -1), inv_cnt
    pl.max_lo_chunks = max(nch[(bb, 0)] for bb in range(nblk))
    pl.max_hi_chunks = max(nch[(bb, 1)] for bb in range(nblk))
    pl.max_blk_mms = max(len(mms) for mms in block_mms)
    return pl


# ----------------------------------------------------------------------------
# Bass program
# ----------------------------------------------------------------------------

def build_program(d: Dims, pl: Plan, debug=False, repeat=1, ablate=()):
    nc = bacc.Bacc("TRN2", target_bir_lowering=False, debug=False,
                   num_devices=d.ncores, num_swdge_queues=4)
    f32, bf16, i16 = mybir.dt.float32, mybir.dt.bfloat16, mybir.dt.int16

    def din(name, shape, dt=f32):
        return nc.dram_tensor(name, shape, dt, kind="ExternalInput").ap()

    xt = din("xt", [d.F, d.shard])
    W1 = din("W1", [d.F, d.H])
    W2 = din("W2", [d.H, d.H], bf16)
    Wc = din("Wc", [d.H, d.C])
    g1 = din("g1", [d.H, 1])
    be1 = din("be1", [d.H, 1])
    g2 = din("g2", [d.H, 1])
    be2 = din("be2", [d.H, 1])
    idx_d = din("idx", [P, pl.tot_slots // 16], i16)
    A_d = din("A", [P, pl.n_mm], bf16)
    dinv_d = din("dinv_pt", [P, d.ntile])
    pool_d = din("pool_pt", [P, d.ntile * d.G])
    invc_d = din("inv_cnt", [d.G, 1])
    bcr_d = din("bc_rep", [d.G, d.C])
    iota_d = din("iota", [P, P], bf16)
    ident_d = din("ident", [P, P])
    out_d = nc.dram_tensor("out", [d.G, d.C], f32, kind="ExternalOutput").ap()
    if debug:
        dbg_agg = nc.dram_tensor("dbg_agg", [P, d.ntile * d.H], f32,
                                 kind="ExternalOutput").ap()
        dbg_h = nc.dram_tensor("dbg_h", [d.H, d.ntile * P], f32,
                               kind="ExternalOutput").ap()
        dbg_y = nc.dram_tensor("dbg_y", [d.npad, d.H], f32,
                               kind="ExternalOutput").ap()

    rg = [list(range(d.ncores))]

    with tile.TileContext(nc) as tc:
        with (
            tc.tile_pool(name="const", bufs=1) as cpool,
            tc.tile_pool(name="work", bufs=2) as wpool,
            tc.tile_pool(name="glo", bufs=3) as gpool_lo,
            tc.tile_pool(name="ghi", bufs=3) as gpool_hi,
            tc.tile_pool(name="spool", bufs=2) as spool,
            tc.tile_pool(name="big", bufs=1) as bigpool,
            tc.tile_pool(name="pseg", bufs=3, space="PSUM") as pseg,
            tc.tile_pool(name="pmm", bufs=2, space="PSUM") as pmm,
            tc.tile_pool(name="ptr", bufs=3, space="PSUM") as ptr,
            tc.tile_pool(name="dram", bufs=1, space="DRAM") as dpool,
        ):
            # ---- load constants ----
            def cload(ap, shape, dt=f32, name=None):
                t = cpool.tile(shape, dt, tag=name)
                nc.sync.dma_start(out=t[:], in_=ap)
                return t

            W1_s = cload(W1[:], [d.F, d.H], name="W1")
            W2_s = cload(W2[:], [d.H, d.H], bf16, name="W2")
            Wc_s = cload(Wc[:], [d.H, d.C], name="Wc")
            g1_s = cload(g1[:], [d.H, 1], name="g1")
            be1_s = cload(be1[:], [d.H, 1], name="be1")
            g2_s = cload(g2[:], [d.H, 1], name="g2")
            be2_s = cload(be2[:], [d.H, 1], name="be2")
            idx_s = cload(idx_d[:], [P, pl.tot_slots // 16], i16, name="idx")
            A_s = cload(A_d[:], [P, pl.n_mm], bf16, name="A")
            dinv_s = cload(dinv_d[:], [P, d.ntile], name="dinv")
            pool_s = cload(pool_d[:], [P, d.ntile * d.G], name="pool")
            invc_s = cload(invc_d[:], [d.G, 1], name="invc")
            bcr_s = cload(bcr_d[:], [d.G, d.C], name="bcr")
            iota_s = cload(iota_d[:], [P, P], bf16, name="iota")
            ident_s = cload(ident_d[:], [P, P], name="ident")

            # pool matrix as bf16 for matmul
            pool_bf = cpool.tile([P, d.ntile * d.G], bf16, tag="poolbf")
            nc.vector.tensor_copy(out=pool_bf[:], in_=pool_s[:])
            eps_s = cpool.tile([d.H, 1], f32, tag="eps")
            nc.vector.memset(eps_s[:], EPS)
            ident_bf = cpool.tile([P, P], bf16, tag="identbf")
            nc.vector.tensor_copy(out=ident_bf[:], in_=ident_s[:])

            # ---- internal DRAM for collectives ----
            y_own = dpool.tile([d.shard_pad, d.H], bf16)
            stats_in = dpool.tile([d.H, 2], f32)
            pool_in = dpool.tile([d.G, d.H], f32)
            pool_out = dpool.tile([d.G, d.H], f32, addr_space="Shared")

            h_fm = None  # feature-major relu'd activations [H, shard]
            gq = [0]

            # node-major table values kept in SBUF for self-loop matmuls
            y_keep = bigpool.tile([P, d.ntile, d.H], bf16, tag="y_keep")
            if d.shard % P:
                nc.vector.memset(y_keep[:, d.ntile - 1, :], 0.0)

            for rep in range(repeat):
              for layer in range(2):
                # fresh Shared collective outputs each round: Tile requires a
                # single writer per Shared DRAM tensor
                y_full = dpool.tile([d.npad, d.H], bf16,
                                    addr_space="Shared" if "sharedyf" in ablate else "Local",
                                    tag=f"y_full_{rep}_{layer}", name="y_full")
                stats_out = dpool.tile([d.H, 2], f32, addr_space="Shared",
                                       tag=f"stats_out_{rep}_{layer}",
                                       name="stats_out")
                # ---------- y = dinv * (X @ W)  (own shard, node-major) ----
                for mt in range(ceil_div(d.shard, 512)):
                    c0 = mt * 512
                    cw = min(512, d.shard - c0)
                    nst = ceil_div(cw, P)
                    if layer == 0:
                        rhs_t = wpool.tile([d.F, 512], f32, tag="xt_t")
                        nc.sync.dma_start(out=rhs_t[:, :cw],
                                          in_=xt[:, c0:c0 + cw])
                        lhsT, rhs_ap = W1_s[:, :], rhs_t[:, :cw]
                    else:
                        lhsT, rhs_ap = W2_s[:, :], h_fm[:, c0:c0 + cw]
                    xw_ps = pmm.tile([d.H, 512], f32, tag="xw")
                    nc.tensor.matmul(out=xw_ps[:, :cw], lhsT=lhsT,
                                     rhs=rhs_ap, start=True, stop=True)
                    xw_sb = wpool.tile([d.H, 512], f32, tag="xw_sb")
                    nc.scalar.copy(out=xw_sb[:, :cw], in_=xw_ps[:, :cw])
                    # transpose 128-node subtiles; dinv scale in ACT evac.
                    # y_keep holds the node-major table values in SBUF for
                    # the self-loop identity matmuls.
                    for st in range(nst):
                        t_global = mt * 4 + st
                        n0 = st * P
                        nw = min(P, cw - n0)
                        tr_ps = ptr.tile([P, d.H], f32, tag="ptr")
                        nc.tensor.transpose(out=tr_ps[:nw, :],
                                            in_=xw_sb[:, n0:n0 + nw],
                                            identity=ident_s[:])
                        nc.scalar.mul(out=y_keep[:nw, t_global, :],
                                      in_=tr_ps[:nw, :],
                                      mul=dinv_s[:nw, t_global:t_global + 1])
                    nc.sync.dma_start(
                        out=y_own[c0:c0 + nst * P, :].rearrange(
                            "(t p) f -> p t f", p=P),
                        in_=y_keep[:, mt * 4:mt * 4 + nst, :])
                if "nogather_collective" in ablate:
                    nc.sync.dma_start(out=y_full[0:d.shard_pad, :],
                                      in_=y_own[:])
                else:
                    nc.gpsimd.collective_compute(
                        "AllGather", mybir.AluOpType.bypass, replica_groups=rg,
                        ins=[y_own.opt()], outs=[y_full.opt()])

                # ---------- gather + segment matmul over blocks ----------
                agg_dm = bigpool.tile([P, d.ntile, d.H], f32, tag="agg_dm")
                if d.shard % P:
                    nc.vector.memset(agg_dm[:, d.ntile - 1, :], 0.0)
                # feature-major pre-BN activations + per-tile stats, filled
                # per block as PSUM windows are evacuated
                h_raw = bigpool.tile([d.H, d.ntile * P], bf16, tag="h_raw")
                s1p = wpool.tile([d.H, d.ntile], f32, tag="s1p")
                s2p = wpool.tile([d.H, d.ntile], f32, tag="s2p")
                scratch = wpool.tile([d.H, P], f32, tag="scr")
                y_lo = y_full[0:d.half, :]
                y_hi = y_full[d.half:d.npad, :]
                for b in range(d.nblk):
                    ntile_b = ceil_div(
                        min(BLOCK_DESTS, d.shard - b * BLOCK_DESTS), P)
                    blk_ps = pseg.tile([P, 4 * d.H], f32, tag="seg")
                    nc.vector.memset(blk_ps[:], 0.0)
                    # self loops: identity matmul against SBUF-resident y
                    for w4 in range(ntile_b):
                        t_global = 4 * b + w4
                        nc.tensor.matmul(
                            out=blk_ps[:, w4 * d.H:(w4 + 1) * d.H],
                            lhsT=ident_bf[:],
                            rhs=y_keep[:, t_global, :],
                            start=False, stop=False,
                            skip_group_check=True,
                        )
                    # gather the two half-groups of this block
                    gtiles = {}
                    for hf, gpool, ysrc in ((0, gpool_lo, y_lo),
                                            (1, gpool_hi, y_hi)):
                        gcnt = pl.nch[(b, hf)]
                        g0 = pl.gstart[(b, hf)]
                        if gcnt == 0 or "nodmagather" in ablate:
                            gtiles[hf] = None
                            continue
                        mgc = pl.max_lo_chunks if hf == 0 else pl.max_hi_chunks
                        gt = gpool.tile([P, mgc, d.H], bf16, tag=f"g{hf}")
                        npc = 1 if "npc1" in ablate else 2
                        bnds = [gcnt * i // npc for i in range(npc + 1)]
                        for pc0, pc1 in zip(bnds, bnds[1:]):
                            if pc1 <= pc0:
                                continue
                            ns_p = (pc1 - pc0) * P
                            s0 = g0 + pc0 * P
                            nc.gpsimd.dma_gather(
                                out_ap=gt[:, pc0:pc1, :],
                                in_ap=ysrc,
                                idxs_ap=idx_s[:, s0 // 16:(s0 + ns_p) // 16],
                                num_idxs=ns_p,
                                num_idxs_reg=ns_p,
                                elem_size=d.H,
                                single_packet=False,
                                queue_num=gq[0] % 4,
                            )
                            gq[0] += 1
                        gtiles[hf] = gt
                    # build S for all segment matmuls of this block
                    mms = pl.block_mms[b]
                    if mms and "nosegmm" not in ablate \
                            and "nodmagather" not in ablate:
                        a0 = mms[0][3]
                        a1 = mms[-1][3] + 1
                        nmm = a1 - a0
                        S_t = spool.tile([P, pl.max_blk_mms, P], bf16,
                                         tag="S")
                        a_b = A_s[:, a0:a1].unsqueeze(2).broadcast_to(
                            [P, nmm, P])
                        i_b = iota_s[:].unsqueeze(1).broadcast_to([P, nmm, P])
                        nc.vector.tensor_tensor(out=S_t[:, :nmm, :], in0=a_b,
                                                in1=i_b,
                                                op=mybir.AluOpType.is_equal)
                        for hf, lc, jj, acol in mms:
                            nc.tensor.matmul(
                                out=blk_ps[:, jj * d.H:(jj + 1) * d.H],
                                lhsT=S_t[:, acol - a0, :],
                                rhs=gtiles[hf][:, lc, :],
                                start=False, stop=False,
                                skip_group_check=True,
                            )
                    # evacuate: dest-major agg with dinv scaling, then
                    # transpose to feature-major + accumulate BN stats
                    for w4 in range(ntile_b):
                        t_global = 4 * b + w4
                        nw = min(P, d.shard - t_global * P)
                        nc.scalar.mul(
                            out=agg_dm[:nw, t_global, :],
                            in_=blk_ps[:nw, w4 * d.H:(w4 + 1) * d.H],
                            mul=dinv_s[:nw, t_global:t_global + 1])
                        tr_ps = ptr.tile([d.H, P], f32, tag="ptr")
                        nc.tensor.transpose(out=tr_ps[:, :],
                                            in_=agg_dm[:, t_global, :],
                                            identity=ident_s[:])
                        nc.scalar.activation(
                            out=h_raw[:, t_global * P:(t_global + 1) * P],
                            in_=tr_ps[:],
                            func=mybir.ActivationFunctionType.Copy,
                            accum_out=s1p[:, t_global:t_global + 1])
                        nc.scalar.activation(
                            out=scratch[:], in_=tr_ps[:],
                            func=mybir.ActivationFunctionType.Square,
                            accum_out=s2p[:, t_global:t_global + 1])

                # ---------- BN stats reduce + exchange ----------
                stats_sb = wpool.tile([d.H, 2], f32, tag="stats")
                nc.vector.tensor_reduce(out=stats_sb[:, 0:1], in_=s1p[:],
                                        axis=mybir.AxisListType.X,
                                        op=mybir.AluOpType.add)
                nc.vector.tensor_reduce(out=stats_sb[:, 1:2], in_=s2p[:],
                                        axis=mybir.AxisListType.X,
                                        op=mybir.AluOpType.add)
                nc.sync.dma_start(out=stats_in[:], in_=stats_sb[:])
                if "nostatsar" in ablate:
                    nc.sync.dma_start(out=stats_out[:], in_=stats_in[:])
                else:
                    nc.gpsimd.collective_compute(
                        "AllReduce", mybir.AluOpType.add, replica_groups=rg,
                        ins=[stats_in.opt()], outs=[stats_out.opt()])
                stats_g = wpool.tile([d.H, 2], f32, tag="statsg")
                nc.sync.dma_start(out=stats_g[:], in_=stats_out[:])
                # mean/var -> scale/bias
                mv = wpool.tile([d.H, 6], f32, tag="mv")
                inv_n = 1.0 / d.N
                nc.vector.tensor_scalar(out=mv[:, 0:1], in0=stats_g[:, 0:1],
                                        scalar1=inv_n, scalar2=None,
                                        op0=mybir.AluOpType.mult)  # mean
                nc.vector.tensor_scalar(out=mv[:, 1:2], in0=stats_g[:, 1:2],
                                        scalar1=inv_n, scalar2=None,
                                        op0=mybir.AluOpType.mult)  # E[x^2]
                nc.vector.tensor_tensor(out=mv[:, 2:3], in0=mv[:, 0:1],
                                        in1=mv[:, 0:1],
                                        op=mybir.AluOpType.mult)   # mean^2
                nc.vector.tensor_tensor(out=mv[:, 2:3], in0=mv[:, 1:2],
                                        in1=mv[:, 2:3],
                                        op=mybir.AluOpType.subtract)  # var
                nc.scalar.activation(out=mv[:, 3:4], in_=mv[:, 2:3],
                                     func=mybir.ActivationFunctionType.Sqrt,
                                     bias=eps_s[:])                # std
                nc.vector.reciprocal(out=mv[:, 4:5], in_=mv[:, 3:4])
                gg = g1_s if layer == 0 else g2_s
                bb = be1_s if layer == 0 else be2_s
                nc.vector.tensor_tensor(out=mv[:, 4:5], in0=mv[:, 4:5],
                                        in1=gg[:], op=mybir.AluOpType.mult)
                # bias = be - mean*scale
                nc.vector.tensor_tensor(out=mv[:, 5:6], in0=mv[:, 0:1],
                                        in1=mv[:, 4:5],
                                        op=mybir.AluOpType.mult)
                nc.vector.tensor_tensor(out=mv[:, 5:6], in0=bb[:],
                                        in1=mv[:, 5:6],
                                        op=mybir.AluOpType.subtract)
                if debug and layer == 0:
                    for t in range(d.ntile * d.ncores):
                        dbg_y_bf = wpool.tile([P, d.H], bf16, tag="dbgybf")
                        dbg_y_sb = wpool.tile([P, d.H], f32, tag="dbgy")
                        nc.sync.dma_start(out=dbg_y_bf[:],
                                          in_=y_full[t * P:(t + 1) * P, :])
                        nc.vector.tensor_copy(out=dbg_y_sb[:], in_=dbg_y_bf[:])
                        nc.sync.dma_start(out=dbg_y[t * P:(t + 1) * P, :],
                                          in_=dbg_y_sb[:])
                    nc.sync.dma_start(
                        out=dbg_agg[:],
                        in_=agg_dm[:].rearrange("p t f -> p (t f)"))
                h_fm = bigpool.tile([d.H, d.ntile * P], bf16, tag="h_fm")
                nc.scalar.activation(out=h_fm[:], in_=h_raw[:],
                                     func=mybir.ActivationFunctionType.Relu,
                                     scale=mv[:, 4:5], bias=mv[:, 5:6])

            if debug:
                dbg_h_sb = wpool.tile([d.H, d.ntile * P], f32, tag="dbgh")
                nc.vector.tensor_copy(out=dbg_h_sb[:], in_=h_fm[:])
                nc.sync.dma_start(out=dbg_h[:], in_=dbg_h_sb[:])
            # ---------- pooling ----------
            # node-major h2 tiles via transpose, then matmul with pool matrix
            pool_ps = ptr.tile([d.G, d.H], f32, tag="ptr")
            for t in range(d.ntile):
                tr_ps = ptr.tile([P, d.H], bf16, tag="ptr")
                nc.tensor.transpose(out=tr_ps[:, :],
                                    in_=h_fm[:, t * P:(t + 1) * P],
                                    identity=ident_bf[:])
                h_dm = wpool.tile([P, d.H], bf16, tag="h_dm")
                nc.scalar.copy(out=h_dm[:], in_=tr_ps[:])
                nc.tensor.matmul(
                    out=pool_ps[:, :],
                    lhsT=pool_bf[:, t * d.G:(t + 1) * d.G],
                    rhs=h_dm[:],
                    start=(t == 0), stop=(t == d.ntile - 1))
            pool_sb = wpool.tile([d.G, d.H], f32, tag="poolsb")
            nc.vector.tensor_scalar(out=pool_sb[:], in0=pool_ps[:],
                                    scalar1=invc_s[:], scalar2=None,
                                    op0=mybir.AluOpType.mult)
            nc.sync.dma_start(out=pool_in[:], in_=pool_sb[:])
            if "nopoolar" in ablate:
                nc.sync.dma_start(out=pool_out[:], in_=pool_in[:])
            else:
                nc.gpsimd.collective_compute(
                    "AllReduce", mybir.AluOpType.add, replica_groups=rg,
                    ins=[pool_in.opt()], outs=[pool_out.opt()])
            pooled = wpool.tile([d.G, d.H], f32, tag="pooled")
            nc.sync.dma_start(out=pooled[:], in_=pool_out[:])
            # transpose pooled -> [H, G]
            pooled_t_ps = ptr.tile([d.H, d.G], f32, tag="ptr")
            nc.tensor.transpose(out=pooled_t_ps[:, :], in_=pooled[:],
                                identity=ident_s[:d.G, :d.G])
            pooled_t = wpool.tile([d.H, d.G], f32, tag="pooledtsb")
            nc.scalar.copy(out=pooled_t[:], in_=pooled_t_ps[:])
            out_ps = ptr.tile([d.G, d.C], f32, tag="ptr")
            nc.tensor.matmul(out=out_ps[:], lhsT=pooled_t[:], rhs=Wc_s[:],
                             start=True, stop=True)
            out_sb = wpool.tile([d.G, d.C], f32, tag="outsb")
            nc.vector.tensor_tensor(out=out_sb[:], in0=out_ps[:],
                                    in1=bcr_s[:], op=mybir.AluOpType.add)
            nc.scalar.activation(out=out_sb[:], in_=out_sb[:],
                                 func=mybir.ActivationFunctionType.Sigmoid)
            nc.sync.dma_start(out=out_d[:], in_=out_sb[:])

    nc.compile()
    return nc


# ----------------------------------------------------------------------------
# Entry point
# ----------------------------------------------------------------------------

def make_in_maps(d: Dims, pl: Plan, inputs):
    x = np.asarray(inputs["x"], np.float32)
    W1 = np.asarray(inputs["W1"], np.float32)
    W2 = np.asarray(inputs["W2"], np.float32)
    Wc = np.asarray(inputs["Wc"], np.float32)
    g1 = np.asarray(inputs["g1"], np.float32).reshape(d.H, 1)
    be1 = np.asarray(inputs["be1"], np.float32).reshape(d.H, 1)
    g2 = np.asarray(inputs["g2"], np.float32).reshape(d.H, 1)
    be2 = np.asarray(inputs["be2"], np.float32).reshape(d.H, 1)
    bc = np.asarray(inputs["bc"], np.float32)
    xt = np.ascontiguousarray(x.T)
    iota = np.tile(np.arange(P, dtype=np.float32), (P, 1)).astype(BF16)
    ident = np.eye(P, dtype=np.float32)
    bc_rep = np.tile(bc.reshape(1, d.C), (d.G, 1)).astype(np.float32)
    in_maps = []
    for k in range(d.ncores):
        in_maps.append({
            "xt": np.ascontiguousarray(xt[:, k * d.shard:(k + 1) * d.shard]),
            "W1": W1, "W2": W2.astype(BF16), "Wc": Wc,
            "g1": g1, "be1": be1, "g2": g2, "be2": be2,
            "idx": pl.idx_wrapped[k],
            "A": np.ascontiguousarray(pl.A_pt[k]),
            "dinv_pt": pl.dinv_pt[k],
            "pool_pt": pl.pool_pt[k],
            "inv_cnt": pl.inv_cnt,
            "bc_rep": bc_rep,
            "iota": iota,
            "ident": ident,
        })
    return in_maps


def kernel(**inputs) -> np.ndarray:
    d = Dims()
    edge_index = np.asarray(inputs["edge_index"], np.int64)
    batch = np.asarray(inputs["batch"], np.int64)
    pl = make_plan(d, edge_index, batch)
    nc = build_program(d, pl)
    in_maps = make_in_maps(d, pl, inputs)
    res = run_bass_kernel_spmd(nc, in_maps, core_ids=list(range(d.ncores)))
    return np.asarray(res.results[0]["out"], np.float32)

